# revision 4
# baseline (speedup 1.0000x reference)
"""BertAttention (T5-style relative-position bias) Trainium2 Bass kernel.

Strategy (8-way tensor parallel over heads, 2 heads/core):
  - Host pre-transposes hidden -> hT [HID, B*S] so QKV projection produces
    qkvT [feat, tokens] directly (feat on partitions).
  - Per core: w_qkv column slice for its 2 heads, ordered [Q_h0|Q_h1|K_h0|K_h1|V_h0|V_h1],
    Q columns pre-scaled by 1/sqrt(HD).
  - Scores computed transposed: S^T[k, q] = (K^T)^T-slice matmuls; T5 bias is added
    inside PSUM via an anti-diagonal (flip) matmul whose rhs is a Toeplitz DMA read
    of the (reversed) per-head expanded bias table; softmax without max-subtraction
    (scores are bounded, |s| < ~4); denominator via ones-column appended to V.
  - PV gives ctx^T [d, q]; normalize by reciprocal of the ones-row.
  - AllToAll reshards ctx^T from head-split to token-split; dense is computed
    transposed (out^T[e, t]) so b_dense is a per-partition bias.
  - Host reassembles out^T column chunks, transposes, reshapes.

All big matmuls run as float32r (full PE rate, ~1e-4 relative precision).
"""
import sys
import math

sys.path.insert(0, "/opt/trn_rl_repo")

import numpy as np
import ml_dtypes

import concourse.bass as bass
import concourse.bacc as bacc
import concourse.tile as tile
import concourse.mybir as mybir
from concourse.bass_utils import run_bass_kernel_spmd
from concourse.masks import make_identity

F32 = mybir.dt.float32
F32R = mybir.dt.float32r
BF16 = mybir.dt.bfloat16
Exp = mybir.ActivationFunctionType.Exp
ADD = mybir.AluOpType.add
MULT = mybir.AluOpType.mult

B, S, HID = 2, 2048, 1024
NH, HD = 16, 64
NB, MAXD = 32, 128
N_CORES = 8
HPC = NH // N_CORES          # heads per core = 2
T = B * S                    # 4096 flat tokens
FEAT = 3 * HPC * HD          # 384 qkv features per core
TC = T // 512                # 8 token chunks of 512
KTILES = S // 128            # 16 k tiles per batch
QCH = S // 512               # 4 q chunks of 512 per batch
TW = 4096                    # padded width of expanded bias table (indices 0..4094 used)


def _bucket_map_rev():
    """rev[z] = bucket(2047 - z) for z in [0, 4094], T5 bidirectional buckets."""
    rel = (2047 - np.arange(TW - 1)).astype(np.int64)   # k - q
    nb = NB // 2                                        # 16
    base = np.where(rel > 0, nb, 0)
    r = np.abs(rel)
    max_exact = nb // 2                                 # 8
    is_small = r < max_exact
    tmp = np.log(np.maximum(r, 1).astype(np.float32) / np.float32(max_exact))
    large = tmp / np.float32(math.log(MAXD / max_exact)) * np.float32(nb - max_exact)
    large_i = max_exact + large.astype(np.int32)
    large_i = np.minimum(large_i, nb - 1)
    return (base + np.where(is_small, r, large_i)).astype(np.int32)  # [4095]


def _build_program():
    nc = bacc.Bacc("TRN2", target_bir_lowering=False, debug=False,
                   enable_asserts=True, num_devices=N_CORES)

    hT_d = nc.dram_tensor("hT", [HID, T], F32R, kind="ExternalInput")
    wq_d = nc.dram_tensor("wq", [HID, FEAT], F32R, kind="ExternalInput")
    bq_d = nc.dram_tensor("bq", [FEAT, 1], F32, kind="ExternalInput")
    wd_d = nc.dram_tensor("wd", [HID, HID], F32R, kind="ExternalInput")
    bd_d = nc.dram_tensor("bd", [HID, 1], F32, kind="ExternalInput")
    tT_d = nc.dram_tensor("tT", [NB, HPC], F32R, kind="ExternalInput")
    oh_d = nc.dram_tensor("oh", [NB, TW], F32R, kind="ExternalInput")
    out_d = nc.dram_tensor("outT", [HID, T // N_CORES], F32, kind="ExternalOutput")

    with tile.TileContext(nc) as tc:
        with tc.tile_pool(name="const", bufs=1) as cst, \
             tc.tile_pool(name="big", bufs=1) as big, \
             tc.tile_pool(name="dram", bufs=1, space="DRAM") as dram:

            # ---------------- constants ----------------
            ident_f = cst.tile([128, 128], F32, tag="identf")
            make_identity(nc, ident_f[:])
            identr = cst.tile([128, 128], F32R, tag="identr")
            nc.vector.tensor_copy(identr[:], ident_f[:])
            jmat = cst.tile([128, 128], BF16, tag="jmat")
            nc.gpsimd.memset(jmat[:], 0.0)
            nc.gpsimd.affine_select(out=jmat[:], in_=jmat[:],
                                    compare_op=mybir.AluOpType.not_equal,
                                    fill=1.0, base=-127, channel_multiplier=1,
                                    pattern=[[1, 128]])
            ones_f = cst.tile([128, 1], F32, tag="ones")
            nc.gpsimd.memset(ones_f[:], 1.0)
            bq_sb = cst.tile([128, 3, 1], F32, tag="bq")
            nc.sync.dma_start(bq_sb[:], bq_d[:, :].rearrange("(m p) o -> p m o", p=128))
            bd_sb = cst.tile([128, 8, 1], F32, tag="bd")
            nc.sync.dma_start(bd_sb[:], bd_d[:, :].rearrange("(e p) o -> p e o", p=128))

            # persistent tensors
            QT = big.tile([128, T], F32R, tag="QT")     # rows 0-63 h0, 64-127 h1
            KT = big.tile([128, T], F32R, tag="KT")
            Vaug = big.tile([128, T // 128, 130], F32R, tag="Vaug")
            ctxT = big.tile([128, T], F32R, tag="ctxT")

            # ---------------- expanded bias table (device-side gather) ----------------
            trev = dram.tile([HPC, TW], BF16)
            with tc.tile_pool(name="txp", bufs=2, space="PSUM") as txp, \
                 tc.tile_pool(name="txs", bufs=1) as txs:
                tT_sb = txs.tile([NB, HPC], F32R, tag="tT")
                nc.sync.dma_start(tT_sb[:], tT_d[:, :])
                oh_sb = txs.tile([NB, TW], F32R, tag="oh")
                nc.sync.dma_start(oh_sb[:], oh_d[:, :])
                trev_sb = txs.tile([HPC, TW], BF16, tag="trevsb")
                for i in range(TW // 512):
                    tx_ps = txp.tile([HPC, 512], F32, tag="tx")
                    nc.tensor.matmul(tx_ps[:], tT_sb[:], oh_sb[:, i * 512:(i + 1) * 512],
                                     start=True, stop=True)
                    nc.vector.tensor_copy(trev_sb[:, i * 512:(i + 1) * 512], tx_ps[:])
                nc.sync.dma_start(trev[:], trev_sb[:])

            # ---------------- QKV projection ----------------
            with tc.tile_pool(name="wqp", bufs=1) as wqp, \
                 tc.tile_pool(name="htp", bufs=12) as htp, \
                 tc.tile_pool(name="vtp", bufs=1) as vtp, \
                 tc.tile_pool(name="qkvps", bufs=3, space="PSUM") as qkvps:
                wq_sb = wqp.tile([128, 8, FEAT], F32R, tag="wq")
                nc.sync.dma_start(wq_sb[:], wq_d[:, :].rearrange("(j p) f -> p j f", p=128))
                VT = vtp.tile([128, T], F32R, tag="VT")
                dests = (QT, KT, VT)
                for tci in range(TC):
                    hts = []
                    for kt in range(8):
                        ht = htp.tile([128, 512], F32R, tag="ht")
                        nc.sync.dma_start(
                            ht[:], hT_d[128 * kt:128 * (kt + 1), 512 * tci:512 * (tci + 1)])
                        hts.append(ht)
                    for m in range(3):
                        ps = qkvps.tile([128, 512], F32, tag="qkv")
                        for kt in range(8):
                            nc.tensor.matmul(ps[:], wq_sb[:, kt, m * 128:(m + 1) * 128],
                                             hts[kt][:], start=(kt == 0), stop=(kt == 7))
                        nc.vector.tensor_tensor(
                            dests[m][:, 512 * tci:512 * (tci + 1)], ps[:],
                            bq_sb[:, m, 0:1].to_broadcast([128, 512]), ADD)

                # ---- V transpose into Vaug (+ ones cols 64 and 129) ----
                with tc.tile_pool(name="trps", bufs=3, space="PSUM") as trps:
                    for t in range(T // 128):
                        vslot = Vaug[:, t, :].rearrange("p (g c) -> p g c", c=65)
                        nc.vector.tensor_copy(
                            vslot[:, :, 64:65],
                            ones_f[:, 0:1].to_broadcast([128, 2, 1]))
                        tp = trps.tile([128, 128], F32R, tag="tr")
                        nc.tensor.transpose(tp[:], VT[:, 128 * t:128 * (t + 1)], identr[:])
                        # cols 0-63 -> Vaug[:,t,0:64]; cols 64-127 -> Vaug[:,t,65:129]
                        nc.vector.tensor_copy(
                            vslot[:, :, 0:64],
                            tp[:].rearrange("p (g c) -> p g c", c=64))

            # ---------------- attention ----------------
            with tc.tile_pool(name="rp", bufs=34) as rp, \
                 tc.tile_pool(name="expp", bufs=3) as expp, \
                 tc.tile_pool(name="nrm", bufs=4) as nrm, \
                 tc.tile_pool(name="sps", bufs=2, space="PSUM") as sps, \
                 tc.tile_pool(name="cps", bufs=2, space="PSUM") as cps:
                for qc in range(QCH):
                    q0 = qc * 512
                    # Toeplitz bias tiles for this q chunk (shared across batches)
                    rtiles = {}
                    for kt in range(KTILES):
                        for h in range(HPC):
                            r = rp.tile([128, 512], BF16, tag="rt")
                            src = bass.AP(trev.tensor,
                                          trev.offset + h * TW + (1920 - kt * 128 + q0),
                                          [[1, 128], [1, 512]])
                            nc.sync.dma_start(r[:], src)
                            rtiles[(kt, h)] = r
                    for b in range(B):
                        qg = b * S + q0
                        ctx_ps = [cps.tile([65, 512], F32, tag="ctx", name=f"ctx{h}_{b}_{qc}")
                                  for h in range(HPC)]
                        for kt in range(KTILES):
                            kg = b * S + kt * 128
                            s_ps = sps.tile([128, 1024], F32, tag="S")
                            for h in range(HPC):
                                ssl = s_ps[:, 512 * h:512 * (h + 1)]
                                nc.tensor.matmul(ssl, KT[64 * h:64 * h + 64, kg:kg + 128],
                                                 QT[64 * h:64 * h + 64, qg:qg + 512],
                                                 start=True, stop=False)
                                nc.tensor.matmul(ssl, jmat[:], rtiles[(kt, h)][:],
                                                 start=False, stop=True)
                            es = expp.tile([128, 1024], F32R, tag="es")
                            nc.scalar.activation(es[:], s_ps[:], Exp)
                            for h in range(HPC):
                                nc.tensor.matmul(ctx_ps[h][:],
                                                 Vaug[:, b * KTILES + kt, 65 * h:65 * h + 65],
                                                 es[:, 512 * h:512 * (h + 1)],
                                                 start=(kt == 0), stop=(kt == KTILES - 1))
                        for h in range(HPC):
                            recip = nrm.tile([1, 512], F32, tag="recip")
                            nc.vector.reciprocal(recip[:], ctx_ps[h][64:65, :])
                            rbb = nrm.tile([64, 512], F32, tag="rbb")
                            nc.gpsimd.partition_broadcast(rbb[:], recip[:])
                            nc.vector.tensor_tensor(
                                ctxT[64 * h:64 * h + 64, qg:qg + 512],
                                ctx_ps[h][0:64, :], rbb[:], MULT)

            # ---------------- all-to-all (head-split -> token-split) ----------------
            a2a_in = dram.tile([HID, T // N_CORES], F32R)
            a2a_out = dram.tile([HID, T // N_CORES], F32R)
            nc.sync.dma_start(a2a_in[:].rearrange("(j p) t -> p j t", p=128),
                              ctxT[:].rearrange("p (j t) -> p j t", t=512))
            nc.gpsimd.collective_compute(
                "AllToAll", mybir.AluOpType.bypass,
                replica_groups=[list(range(N_CORES))],
                ins=[a2a_in[:].opt()], outs=[a2a_out[:].opt()])

            # ---------------- dense (out^T = W^T-slices @ ctx_full) ----------------
            with tc.tile_pool(name="wdp", bufs=1) as wdp, \
                 tc.tile_pool(name="dps", bufs=3, space="PSUM") as dps:
                wd_sb = wdp.tile([128, 8, HID], F32R, tag="wd")
                nc.sync.dma_start(wd_sb[:], wd_d[:, :].rearrange("(j p) e -> p j e", p=128))
                ctxf = wdp.tile([128, 8, 512], F32R, tag="ctxf")
                nc.sync.dma_start(ctxf[:], a2a_out[:].rearrange("(j p) t -> p j t", p=128))
                outT_sb = wdp.tile([128, 8, 512], F32, tag="outT")
                for e in range(8):
                    ps = dps.tile([128, 512], F32, tag="d")
                    for j in range(8):
                        nc.tensor.matmul(ps[:], wd_sb[:, j, 128 * e:128 * (e + 1)],
                                         ctxf[:, j, :], start=(j == 0), stop=(j == 7))
                    nc.vector.tensor_tensor(outT_sb[:, e, :], ps[:],
                                            bd_sb[:, e, 0:1].to_broadcast([128, 512]), ADD)
                nc.sync.dma_start(out_d[:, :].rearrange("(e p) t -> p e t", p=128),
                                  outT_sb[:])

    nc.compile()
    return nc


_NC_CACHE = None


def _get_program():
    global _NC_CACHE
    if _NC_CACHE is None:
        _NC_CACHE = _build_program()
    return _NC_CACHE


def kernel(hidden_states, w_qkv, b_qkv, w_dense, b_dense, rel_attn_table):
    hidden_states = np.asarray(hidden_states, dtype=np.float32)
    w_qkv = np.asarray(w_qkv, dtype=np.float32)
    b_qkv = np.asarray(b_qkv, dtype=np.float32)
    w_dense = np.asarray(w_dense, dtype=np.float32)
    b_dense = np.asarray(b_dense, dtype=np.float32)
    rel_attn_table = np.asarray(rel_attn_table, dtype=np.float32)

    hT = np.ascontiguousarray(hidden_states.reshape(T, HID).T)   # [HID, T]

    # one-hot bucket map (constant, shapes only)
    bm = _bucket_map_rev()                                       # [4095]
    oh = np.zeros((NB, TW), dtype=np.float32)
    oh[bm, np.arange(TW - 1)] = 1.0

    scale = np.float32(1.0 / math.sqrt(HD))
    in_maps = []
    for c in range(N_CORES):
        ha, hb = HPC * c, HPC * c + 1
        cols = []
        bias = []
        for blk, sc in ((0, scale), (1, np.float32(1.0)), (2, np.float32(1.0))):
            for h in (ha, hb):
                sl = slice(blk * HID + h * HD, blk * HID + (h + 1) * HD)
                cols.append(w_qkv[:, sl] * sc)
                bias.append(b_qkv[sl] * sc)
        wq_c = np.ascontiguousarray(np.concatenate(cols, axis=1))        # [HID, 384]
        bq_c = np.concatenate(bias).reshape(FEAT, 1).astype(np.float32)
        in_maps.append({
            "hT": hT,
            "wq": wq_c,
            "bq": bq_c,
            "wd": w_dense,
            "bd": b_dense.reshape(HID, 1),
            "tT": np.ascontiguousarray(rel_attn_table[ha:hb + 1].T),     # [32, 2]
            "oh": oh,
        })

    nc = _get_program()
    res = run_bass_kernel_spmd(nc, in_maps, core_ids=list(range(N_CORES)))
    outT = np.concatenate([res.results[c]["outT"] for c in range(N_CORES)], axis=1)
    return np.ascontiguousarray(outT.T).reshape(B, S, HID)
